# revision 5
# baseline (speedup 1.0000x reference)
"""Pairwise squared L2 distance (retrieval KNN) on 8 TRN2 NeuronCores.

dist[i, j] = ||x_i||^2 + ||y_j||^2 - 2 * <x_i, y_j>

Sharding: rows of x are split across the 8 cores (data-parallel over n);
y is replicated. Each core computes a [1024, 8192] slab of the distance
matrix.

Memory-roofline design (rel tol 2e-2 allows 16-bit end to end):
- Single fp16 matmul for the cross term (x pre-scaled by -2 host-side,
  so PSUM = -2<x,y>). ~1e-3 max rel err, 20x inside tolerance.
- Output stored as fp16 (~17 MB/core instead of 34), host casts back to
  fp32 after the gather. Device HBM traffic ~20 MB/core -> ~57 us
  roofline at 358 GB/s per core; every engine is kept below that.
- The epilogue (+xsq[p] +ysq[j], 8.4M elems twice) would exceed the DMA
  floor on ScalarE+VectorE alone, so it is split by column half:
  cols 0:4096   op1 = psum + xsq[p] (ScalarE bias / VectorE
                tensor_scalar), op2 = + ysq[j] (VectorE fp16
                tensor_tensor, 4096 wide) with a host-loaded ysq
                broadcast tile;
  cols 4096:8192  both norms folded into the GEMM by a second
                accumulate matmul with contraction-4 fp16 hi/lo rows
                {ysq_h, ysq_l} x {xsq_h, xsq_l} (norm err ~1e-4, and
                K=4 matmuls stream at full rate), leaving a single
                PSUM -> SBUF fp16 copy.
  That lands ScalarE ~43 us, VectorE ~42 us, PE ~46 us, all < DMA.
- A dummy ACTIVATE at the top pulls the one-time ~2.7 us ACT table
  load into the DMA load phase.
"""

import numpy as np

import concourse.bass as bass
import concourse.mybir as mybir
import concourse.tile as tile
from concourse import bacc
from concourse.bass import ts
from concourse.bass_utils import run_bass_kernel_spmd

N, M, D = 8192, 8192, 128
NCORES = 8
SLAB = N // NCORES  # 1024 rows of x per core
P = 128  # partitions / m-chunk height
MCH = SLAB // P  # 8 m-chunks per core
NT = 512  # matmul free-dim tile (one fp32 PSUM bank)
GW = 4  # n-chunks per PSUM group (4 banks = 8 KiB/partition)
GCOLS = GW * NT  # 2048
NG = M // GCOLS  # 4 column groups
PCOLS = 2 * GCOLS  # 4096: op2/store width (two groups)
HB = M // 2  # 4096: cols 0:HB use op1/op2, cols HB:M use the norm fold

_f32 = mybir.dt.float32
_f16 = mybir.dt.float16
_IDENT = mybir.ActivationFunctionType.Identity

# Unit ops (op1 or fold-copy; 32 of ~2 us) on VectorE for these indices,
# ScalarE otherwise: 22 ACT / 10 DVE balances both at ~43 us.
_DVE_UNIT = {2, 5, 8, 11, 14, 17, 20, 23, 26, 29}

_compiled_nc = None


def _build():
    """Build + compile the single-core Bass program (SPMD across 8 cores)."""
    nc = bacc.Bacc(
        "TRN2",
        target_bir_lowering=False,
        debug=False,
        enable_asserts=False,
        num_devices=NCORES,
    )
    xh = nc.dram_tensor("xh", [D, SLAB], _f16, kind="ExternalInput").ap()
    yh = nc.dram_tensor("yh", [D, M], _f16, kind="ExternalInput").ap()
    xsq = nc.dram_tensor("xsq", [P, MCH], _f32, kind="ExternalInput").ap()
    ysqb = nc.dram_tensor("ysqb", [P, HB], _f16, kind="ExternalInput").ap()
    normw = nc.dram_tensor("normw", [4, SLAB], _f16, kind="ExternalInput").ap()
    normv = nc.dram_tensor("normv", [4, M - HB], _f16, kind="ExternalInput").ap()
    dist = nc.dram_tensor("dist", [SLAB, M], _f16, kind="ExternalOutput").ap()

    with tile.TileContext(nc) as tc:
        with (
            tc.tile_pool(name="consts", bufs=1) as cpool,
            tc.tile_pool(name="psum", bufs=2, space="PSUM") as pspool,
            tc.tile_pool(name="abuf", bufs=3) as apool,
            tc.tile_pool(name="obuf", bufs=4) as opool,
        ):
            # Warm the ACT spline tables during the load phase.
            dum = cpool.tile([1, 8], _f32)
            nc.vector.memset(dum[:], 0.0)
            dum2 = cpool.tile([1, 8], _f32)
            nc.scalar.activation(dum2[:], dum[:], _IDENT, bias=0.0, scale=1.0)

            # First-block inputs lead so the PE can start ASAP.
            xh_sb = cpool.tile([D, SLAB], _f16)
            nc.sync.dma_start(xh_sb[:], xh[:])
            yh_sb = cpool.tile([D, M], _f16)
            nc.sync.dma_start(yh_sb[:, ts(0, GCOLS)], yh[:, ts(0, GCOLS)])
            nc.sync.dma_start(yh_sb[:, ts(1, GCOLS)], yh[:, ts(1, GCOLS)])
            ysqb_sb = cpool.tile([P, HB], _f16)
            nc.sync.dma_start(ysqb_sb[:], ysqb[:])
            xsq_sb = cpool.tile([P, MCH], _f32)
            nc.sync.dma_start(xsq_sb[:], xsq[:])
            normw_sb = cpool.tile([4, SLAB], _f16)
            nc.sync.dma_start(normw_sb[:], normw[:])
            normv_sb = cpool.tile([4, M - HB], _f16)
            nc.sync.dma_start(normv_sb[:], normv[:])
            nc.sync.dma_start(yh_sb[:, ts(2, GCOLS)], yh[:, ts(2, GCOLS)])
            nc.sync.dma_start(yh_sb[:, ts(3, GCOLS)], yh[:, ts(3, GCOLS)])

            unit = 0
            for gp in range(NG // 2):
                fold = gp == 1
                for mc in range(MCH):
                    xh_w = xh_sb[:, ts(mc, P)]
                    xsq_col = xsq_sb[:, mc : mc + 1]
                    # Both groups' main matmuls first (one stationary load),
                    # then both norm matmuls (one more), then the epilogue.
                    pss = []
                    for half in range(2):
                        g = 2 * gp + half
                        ps = pspool.tile([P, GCOLS], _f32, tag="ps")
                        pss.append(ps)
                        for jj in range(GW):
                            nc.tensor.matmul(
                                ps[:, ts(jj, NT)],
                                xh_w,
                                yh_sb[:, ts(g * GW + jj, NT)],
                                start=True,
                                stop=not fold,
                            )
                    if fold:
                        nw = normw_sb[:, ts(mc, P)]
                        for half in range(2):
                            g = 2 * gp + half
                            for jj in range(GW):
                                nc.tensor.matmul(
                                    pss[half][:, ts(jj, NT)],
                                    nw,
                                    normv_sb[:, ts((g - 2) * GW + jj, NT)],
                                    start=False,
                                    stop=True,
                                )
                        # Epilogue: plain PSUM -> fp16 copy per group.
                        ot = opool.tile([P, PCOLS], _f16, tag="ot")
                        for half in range(2):
                            oh = ot[:, ts(half, GCOLS)]
                            if unit in _DVE_UNIT:
                                nc.vector.tensor_copy(oh, pss[half][:])
                            else:
                                nc.scalar.copy(oh, pss[half][:])
                            unit += 1
                    else:
                        # op1: a = psum + x_sq[p]; op2: out = a + y_sq[j].
                        a4 = apool.tile([P, PCOLS], _f16, tag="a")
                        for half in range(2):
                            ah = a4[:, ts(half, GCOLS)]
                            if unit in _DVE_UNIT:
                                nc.vector.tensor_scalar_add(
                                    ah, pss[half][:], xsq_col
                                )
                            else:
                                nc.scalar.activation(
                                    ah, pss[half][:], _IDENT,
                                    bias=xsq_col, scale=1.0,
                                )
                            unit += 1
                        ot = opool.tile([P, PCOLS], _f16, tag="ot")
                        nc.vector.tensor_add(
                            ot[:], a4[:], ysqb_sb[:, ts(gp, PCOLS)]
                        )
                    nc.sync.dma_start(dist[ts(mc, P), ts(gp, PCOLS)], ot[:])

    nc.compile()
    return nc


def _get_nc():
    global _compiled_nc
    if _compiled_nc is None:
        _compiled_nc = _build()
    return _compiled_nc


def make_in_maps(x: np.ndarray, y: np.ndarray) -> list[dict[str, np.ndarray]]:
    x = np.asarray(x, dtype=np.float32)
    y = np.asarray(y, dtype=np.float32)
    x_sq = np.sum(x * x, axis=1, dtype=np.float32)
    y_sq = np.sum(y * y, axis=1, dtype=np.float32)

    xt2 = np.ascontiguousarray((-2.0 * x).T.astype(np.float16))  # [D, N]
    yt = np.ascontiguousarray(y.T.astype(np.float16))  # [D, M]
    ysqb = np.ascontiguousarray(
        np.broadcast_to(y_sq[:HB].astype(np.float16), (P, HB))
    )

    # fp16 hi/lo splits of the norms for the fold matmul (cols HB:M).
    # Contraction rows: k0: 1*ysq_h[j], k1: 1*ysq_l[j],
    #                   k2: xsq_h[p]*1, k3: xsq_l[p]*1
    ysq_h = y_sq[HB:].astype(np.float16)
    ysq_l = (y_sq[HB:] - ysq_h.astype(np.float32)).astype(np.float16)
    ones_m = np.ones(M - HB, dtype=np.float16)
    normv = np.ascontiguousarray(np.stack([ysq_h, ysq_l, ones_m, ones_m]))
    xsq_h = x_sq.astype(np.float16)
    xsq_l = (x_sq - xsq_h.astype(np.float32)).astype(np.float16)
    ones_n = np.ones(SLAB, dtype=np.float16)

    in_maps = []
    for c in range(NCORES):
        sl = slice(c * SLAB, (c + 1) * SLAB)
        # [P, MCH]: column mc holds x_sq for rows mc*128..mc*128+127
        xsq_in = np.ascontiguousarray(x_sq[sl].reshape(MCH, P).T)
        normw = np.ascontiguousarray(
            np.stack([ones_n, ones_n, xsq_h[sl], xsq_l[sl]])
        )
        in_maps.append(
            {
                "xh": np.ascontiguousarray(xt2[:, sl]),
                "yh": yt,
                "xsq": xsq_in,
                "ysqb": ysqb,
                "normw": normw,
                "normv": normv,
            }
        )
    return in_maps


def kernel(x: np.ndarray, y: np.ndarray, **run_kwargs) -> np.ndarray:
    nc = _get_nc()
    in_maps = make_in_maps(x, y)
    res = run_bass_kernel_spmd(nc, in_maps, core_ids=list(range(NCORES)), **run_kwargs)
    out = np.concatenate(
        [res.results[c]["dist"] for c in range(NCORES)], axis=0
    ).astype(np.float32)
    if run_kwargs:
        kernel.last_results = res
    return out


# revision 6
# speedup vs baseline: 1.1227x; 1.1227x over previous
"""Pairwise squared L2 distance (retrieval KNN) on 8 TRN2 NeuronCores.

dist[i, j] = ||x_i||^2 + ||y_j||^2 - 2 * <x_i, y_j>

Sharding: rows of x are split across the 8 cores (data-parallel over n);
y is replicated. Each core computes a [1024, 8192] slab of the distance
matrix.

Memory-roofline design (rel tol 2e-2 allows 16-bit end to end):
- Single fp16 matmul for the cross term (x pre-scaled by -2 host-side,
  so PSUM = -2<x,y>). ~1e-3 max rel err, 20x inside tolerance. Matmuls
  stay single-instruction accumulation groups: split start/stop (e.g.
  folding norm rows via a second accumulate matmul) halves PE issue
  rate on this silicon, so the norms ride the epilogue instead.
- Output stored as fp16 (~17 MB/core instead of 34), host casts back
  to fp32 after the gather. Device HBM traffic ~20 MB/core -> ~57 us
  roofline at 358 GB/s per core.
- Epilogue work (2 passes over 8.4M elems) is spread over three
  engines so none exceeds the DMA floor:
  op1: a = psum + x_sq[p]  (ScalarE bias-activation for 25 blocks,
       VectorE tensor_scalar for 7)
  op2: out = a + y_sq[j]   (fp16 tensor_tensor, 4096 wide: VectorE for
       13 column pairs, GpSimd for 3)
  The y_sq broadcast tile comes from the host for cols 0:4096 (needed
  in the first ~15 us) and from GpSimd partition_broadcast for cols
  4096:8192 (needed after ~35 us, hiding the ~6 us Q7 library load).
- A dummy ACTIVATE at the top pulls the one-time ~2.7 us ACT table
  load into the DMA load phase.
"""

import numpy as np

import concourse.bass as bass
import concourse.mybir as mybir
import concourse.tile as tile
from concourse import bacc
from concourse.bass import ts
from concourse.bass_utils import run_bass_kernel_spmd

N, M, D = 8192, 8192, 128
NCORES = 8
SLAB = N // NCORES  # 1024 rows of x per core
P = 128  # partitions / m-chunk height
MCH = SLAB // P  # 8 m-chunks per core
NT = 512  # matmul free-dim tile (one fp32 PSUM bank)
GW = 4  # n-chunks per PSUM group (4 banks = 8 KiB/partition)
GCOLS = GW * NT  # 2048
NG = M // GCOLS  # 4 column groups
PCOLS = 2 * GCOLS  # 4096: op2/store width (two groups)
HB = M // 2  # 4096: host-provided half of the ysq broadcast tile

_f32 = mybir.dt.float32
_f16 = mybir.dt.float16
_IDENT = mybir.ActivationFunctionType.Identity

# op1 on VectorE for these block indices (of 32), ScalarE else (25/7).
_DVE_OP1 = {2, 7, 12, 16, 21, 26, 31}
# op2 on GpSimd for these column-pair indices (of 16), VectorE else.
_GPS_OP2 = {5, 9, 13}

_compiled_nc = None


def _build():
    """Build + compile the single-core Bass program (SPMD across 8 cores)."""
    nc = bacc.Bacc(
        "TRN2",
        target_bir_lowering=False,
        debug=False,
        enable_asserts=False,
        num_devices=NCORES,
    )
    xh = nc.dram_tensor("xh", [D, SLAB], _f16, kind="ExternalInput").ap()
    yh = nc.dram_tensor("yh", [D, M], _f16, kind="ExternalInput").ap()
    xsq = nc.dram_tensor("xsq", [P, MCH], _f32, kind="ExternalInput").ap()
    ysqb = nc.dram_tensor("ysqb", [P, HB], _f16, kind="ExternalInput").ap()
    ysqr = nc.dram_tensor("ysqr", [1, M - HB], _f16, kind="ExternalInput").ap()
    dist = nc.dram_tensor("dist", [SLAB, M], _f16, kind="ExternalOutput").ap()

    with tile.TileContext(nc) as tc:
        with (
            tc.tile_pool(name="consts", bufs=1) as cpool,
            tc.tile_pool(name="psum", bufs=2, space="PSUM") as pspool,
            tc.tile_pool(name="abuf", bufs=3) as apool,
            tc.tile_pool(name="obuf", bufs=4) as opool,
        ):
            # Warm the ACT spline tables during the load phase.
            dum = cpool.tile([1, 8], _f32)
            nc.vector.memset(dum[:], 0.0)
            dum2 = cpool.tile([1, 8], _f32)
            nc.scalar.activation(dum2[:], dum[:], _IDENT, bias=0.0, scale=1.0)

            # First-block inputs lead so the PE can start ASAP.
            xh_sb = cpool.tile([D, SLAB], _f16)
            nc.sync.dma_start(xh_sb[:], xh[:])
            yh_sb = cpool.tile([D, M], _f16)
            nc.sync.dma_start(yh_sb[:, ts(0, GCOLS)], yh[:, ts(0, GCOLS)])
            nc.sync.dma_start(yh_sb[:, ts(1, GCOLS)], yh[:, ts(1, GCOLS)])
            ysq_b = cpool.tile([P, M], _f16)
            nc.sync.dma_start(ysq_b[:, 0:HB], ysqb[:])
            xsq_sb = cpool.tile([P, MCH], _f32)
            nc.sync.dma_start(xsq_sb[:], xsq[:])
            ysqr_sb = cpool.tile([1, M - HB], _f16)
            nc.sync.dma_start(ysqr_sb[:], ysqr[:])
            nc.sync.dma_start(yh_sb[:, ts(2, GCOLS)], yh[:, ts(2, GCOLS)])
            nc.sync.dma_start(yh_sb[:, ts(3, GCOLS)], yh[:, ts(3, GCOLS)])

            # ysq_b[p, j] = y_sq[j] for the back half, built on GpSimd
            # (its ~6 us library load hides behind the first column pairs).
            for c in range(2):
                nc.gpsimd.partition_broadcast(
                    ysq_b[:, HB + c * GCOLS : HB + (c + 1) * GCOLS],
                    ysqr_sb[0:1, ts(c, GCOLS)],
                )

            blk = 0
            for gp in range(NG // 2):
                for mc in range(MCH):
                    pair = gp * MCH + mc
                    xh_w = xh_sb[:, ts(mc, P)]
                    xsq_col = xsq_sb[:, mc : mc + 1]
                    a4 = apool.tile([P, PCOLS], _f16, tag="a")
                    for half in range(2):
                        g = 2 * gp + half
                        ps = pspool.tile([P, GCOLS], _f32, tag="ps")
                        for jj in range(GW):
                            nc.tensor.matmul(
                                ps[:, ts(jj, NT)],
                                xh_w,
                                yh_sb[:, ts(g * GW + jj, NT)],
                                start=True,
                                stop=True,
                            )
                        # op1: a = psum + x_sq (per-partition)
                        ah = a4[:, ts(half, GCOLS)]
                        if blk in _DVE_OP1:
                            nc.vector.tensor_scalar_add(ah, ps[:], xsq_col)
                        else:
                            nc.scalar.activation(
                                ah, ps[:], _IDENT, bias=xsq_col, scale=1.0
                            )
                        blk += 1
                    # op2: out = a + y_sq over both groups at once
                    ot = opool.tile([P, PCOLS], _f16, tag="ot")
                    eng = nc.gpsimd if pair in _GPS_OP2 else nc.vector
                    eng.tensor_add(ot[:], a4[:], ysq_b[:, ts(gp, PCOLS)])
                    nc.sync.dma_start(dist[ts(mc, P), ts(gp, PCOLS)], ot[:])

    nc.compile()
    return nc


def _get_nc():
    global _compiled_nc
    if _compiled_nc is None:
        _compiled_nc = _build()
    return _compiled_nc


def make_in_maps(x: np.ndarray, y: np.ndarray) -> list[dict[str, np.ndarray]]:
    x = np.asarray(x, dtype=np.float32)
    y = np.asarray(y, dtype=np.float32)
    x_sq = np.sum(x * x, axis=1, dtype=np.float32)
    y_sq = np.sum(y * y, axis=1, dtype=np.float32)

    xt2 = np.ascontiguousarray((-2.0 * x).T.astype(np.float16))  # [D, N]
    yt = np.ascontiguousarray(y.T.astype(np.float16))  # [D, M]
    ysq16 = y_sq.astype(np.float16)
    ysqb = np.ascontiguousarray(np.broadcast_to(ysq16[:HB], (P, HB)))
    ysqr = np.ascontiguousarray(ysq16[HB:].reshape(1, M - HB))

    in_maps = []
    for c in range(NCORES):
        sl = slice(c * SLAB, (c + 1) * SLAB)
        # [P, MCH]: column mc holds x_sq for rows mc*128..mc*128+127
        xsq_in = np.ascontiguousarray(x_sq[sl].reshape(MCH, P).T)
        in_maps.append(
            {
                "xh": np.ascontiguousarray(xt2[:, sl]),
                "yh": yt,
                "xsq": xsq_in,
                "ysqb": ysqb,
                "ysqr": ysqr,
            }
        )
    return in_maps


def kernel(x: np.ndarray, y: np.ndarray, **run_kwargs) -> np.ndarray:
    nc = _get_nc()
    in_maps = make_in_maps(x, y)
    res = run_bass_kernel_spmd(nc, in_maps, core_ids=list(range(NCORES)), **run_kwargs)
    out = np.concatenate(
        [res.results[c]["dist"] for c in range(NCORES)], axis=0
    ).astype(np.float32)
    if run_kwargs:
        kernel.last_results = res
    return out


# revision 7
# speedup vs baseline: 1.2432x; 1.1073x over previous
"""Pairwise squared L2 distance (retrieval KNN) on 8 TRN2 NeuronCores.

dist[i, j] = ||x_i||^2 + ||y_j||^2 - 2 * <x_i, y_j>

Sharding: rows of x are split across the 8 cores (data-parallel over n);
y is replicated. Each core computes a [1024, 8192] slab of the distance
matrix.

Memory-roofline design (rel tol 2e-2 allows 16-bit end to end):
- Single fp16 matmul for the cross term (x pre-scaled by -2 host-side,
  so PSUM = -2<x,y>). ~1e-3 max rel err, 20x inside tolerance. Matmuls
  stay single-instruction accumulation groups: split start/stop (e.g.
  folding norm rows via a second accumulate matmul) halves PE issue
  rate on this silicon, so the norms ride the epilogue instead.
- Output stored as fp16 (~17 MB/core instead of 34), host casts back
  to fp32 after the gather. Device HBM traffic ~20 MB/core -> ~57 us
  roofline at 358 GB/s per core.
- Epilogue work (2 passes over 8.4M elems) is spread over three
  engines so none exceeds the DMA floor:
  op1: a = psum + x_sq[p]  (ScalarE bias-activation for 26 blocks,
       VectorE tensor_scalar for 6)
  op2: out = a + y_sq[j]   (VectorE fp16 tensor_tensor, 4096 wide;
       GpSimd shares VectorE's SBUF port, so it only builds the late
       half of the ysq broadcast tile)
  The y_sq broadcast tile comes from the host for cols 0:4096 (needed
  in the first ~15 us) and from GpSimd partition_broadcast for cols
  4096:8192 (needed after ~35 us, hiding the ~6 us Q7 library load).
- A dummy ACTIVATE at the top pulls the one-time ~2.7 us ACT table
  load into the DMA load phase.
"""

import numpy as np

import concourse.bass as bass
import concourse.mybir as mybir
import concourse.tile as tile
from concourse import bacc
from concourse.bass import ts
from concourse.bass_utils import run_bass_kernel_spmd

N, M, D = 8192, 8192, 128
NCORES = 8
SLAB = N // NCORES  # 1024 rows of x per core
P = 128  # partitions / m-chunk height
MCH = SLAB // P  # 8 m-chunks per core
NT = 512  # matmul free-dim tile (one fp32 PSUM bank)
GW = 4  # n-chunks per PSUM group (4 banks = 8 KiB/partition)
GCOLS = GW * NT  # 2048
NG = M // GCOLS  # 4 column groups
PCOLS = 2 * GCOLS  # 4096: op2/store width (two groups)
HB = M // 2  # 4096: host-provided half of the ysq broadcast tile

_f32 = mybir.dt.float32
_f16 = mybir.dt.float16
_IDENT = mybir.ActivationFunctionType.Identity

# op1 on VectorE for these block indices (of 32), ScalarE else (25/7).
_DVE_OP1 = {2, 7, 13, 18, 24, 29}

_compiled_nc = None


def _build():
    """Build + compile the single-core Bass program (SPMD across 8 cores)."""
    nc = bacc.Bacc(
        "TRN2",
        target_bir_lowering=False,
        debug=False,
        enable_asserts=False,
        num_devices=NCORES,
    )
    xh = nc.dram_tensor("xh", [D, SLAB], _f16, kind="ExternalInput").ap()
    yh = nc.dram_tensor("yh", [D, M], _f16, kind="ExternalInput").ap()
    xsq = nc.dram_tensor("xsq", [P, MCH], _f32, kind="ExternalInput").ap()
    ysqb = nc.dram_tensor("ysqb", [P, HB], _f16, kind="ExternalInput").ap()
    ysqr = nc.dram_tensor("ysqr", [1, M - HB], _f16, kind="ExternalInput").ap()
    dist = nc.dram_tensor("dist", [SLAB, M], _f16, kind="ExternalOutput").ap()

    with tile.TileContext(nc) as tc:
        with (
            tc.tile_pool(name="consts", bufs=1) as cpool,
            tc.tile_pool(name="psum", bufs=2, space="PSUM") as pspool,
            tc.tile_pool(name="abuf", bufs=3) as apool,
            tc.tile_pool(name="obuf", bufs=4) as opool,
        ):
            # Warm the ACT spline tables during the load phase.
            dum = cpool.tile([1, 8], _f32)
            nc.vector.memset(dum[:], 0.0)
            dum2 = cpool.tile([1, 8], _f32)
            nc.scalar.activation(dum2[:], dum[:], _IDENT, bias=0.0, scale=1.0)

            # First-block inputs lead so the PE can start ASAP.
            xh_sb = cpool.tile([D, SLAB], _f16)
            nc.sync.dma_start(xh_sb[:], xh[:])
            yh_sb = cpool.tile([D, M], _f16)
            nc.sync.dma_start(yh_sb[:, 0:NT], yh[:, 0:NT])
            nc.sync.dma_start(yh_sb[:, NT:GCOLS], yh[:, NT:GCOLS])
            nc.sync.dma_start(yh_sb[:, ts(1, GCOLS)], yh[:, ts(1, GCOLS)])
            ysq_b = cpool.tile([P, M], _f16)
            nc.sync.dma_start(ysq_b[:, 0:HB], ysqb[:])
            xsq_sb = cpool.tile([P, MCH], _f32)
            nc.sync.dma_start(xsq_sb[:], xsq[:])
            ysqr_sb = cpool.tile([1, M - HB], _f16)
            nc.sync.dma_start(ysqr_sb[:], ysqr[:])
            nc.sync.dma_start(yh_sb[:, ts(2, GCOLS)], yh[:, ts(2, GCOLS)])
            nc.sync.dma_start(yh_sb[:, ts(3, GCOLS)], yh[:, ts(3, GCOLS)])

            # ysq_b[p, j] = y_sq[j] for the back half, built on GpSimd
            # (its ~6 us library load hides behind the first column pairs).
            for c in range(2):
                nc.gpsimd.partition_broadcast(
                    ysq_b[:, HB + c * GCOLS : HB + (c + 1) * GCOLS],
                    ysqr_sb[0:1, ts(c, GCOLS)],
                )

            blk = 0
            for gp in range(NG // 2):
                for mc in range(MCH):
                    xh_w = xh_sb[:, ts(mc, P)]
                    xsq_col = xsq_sb[:, mc : mc + 1]
                    a4 = apool.tile([P, PCOLS], _f16, tag="a")
                    for half in range(2):
                        g = 2 * gp + half
                        ps = pspool.tile([P, GCOLS], _f32, tag="ps")
                        for jj in range(GW):
                            nc.tensor.matmul(
                                ps[:, ts(jj, NT)],
                                xh_w,
                                yh_sb[:, ts(g * GW + jj, NT)],
                                start=True,
                                stop=True,
                            )
                        # op1: a = psum + x_sq (per-partition)
                        ah = a4[:, ts(half, GCOLS)]
                        if blk in _DVE_OP1:
                            nc.vector.tensor_scalar_add(ah, ps[:], xsq_col)
                        else:
                            nc.scalar.activation(
                                ah, ps[:], _IDENT, bias=xsq_col, scale=1.0
                            )
                        blk += 1
                    # op2: out = a + y_sq over both groups at once
                    ot = opool.tile([P, PCOLS], _f16, tag="ot")
                    nc.vector.tensor_add(ot[:], a4[:], ysq_b[:, ts(gp, PCOLS)])
                    nc.sync.dma_start(dist[ts(mc, P), ts(gp, PCOLS)], ot[:])

    nc.compile()
    return nc


def _get_nc():
    global _compiled_nc
    if _compiled_nc is None:
        _compiled_nc = _build()
    return _compiled_nc


def make_in_maps(x: np.ndarray, y: np.ndarray) -> list[dict[str, np.ndarray]]:
    x = np.asarray(x, dtype=np.float32)
    y = np.asarray(y, dtype=np.float32)
    x_sq = np.sum(x * x, axis=1, dtype=np.float32)
    y_sq = np.sum(y * y, axis=1, dtype=np.float32)

    xt2 = np.ascontiguousarray((-2.0 * x).T.astype(np.float16))  # [D, N]
    yt = np.ascontiguousarray(y.T.astype(np.float16))  # [D, M]
    ysq16 = y_sq.astype(np.float16)
    ysqb = np.ascontiguousarray(np.broadcast_to(ysq16[:HB], (P, HB)))
    ysqr = np.ascontiguousarray(ysq16[HB:].reshape(1, M - HB))

    in_maps = []
    for c in range(NCORES):
        sl = slice(c * SLAB, (c + 1) * SLAB)
        # [P, MCH]: column mc holds x_sq for rows mc*128..mc*128+127
        xsq_in = np.ascontiguousarray(x_sq[sl].reshape(MCH, P).T)
        in_maps.append(
            {
                "xh": np.ascontiguousarray(xt2[:, sl]),
                "yh": yt,
                "xsq": xsq_in,
                "ysqb": ysqb,
                "ysqr": ysqr,
            }
        )
    return in_maps


def kernel(x: np.ndarray, y: np.ndarray, **run_kwargs) -> np.ndarray:
    nc = _get_nc()
    in_maps = make_in_maps(x, y)
    res = run_bass_kernel_spmd(nc, in_maps, core_ids=list(range(NCORES)), **run_kwargs)
    out = np.concatenate(
        [res.results[c]["dist"] for c in range(NCORES)], axis=0
    ).astype(np.float32)
    if run_kwargs:
        kernel.last_results = res
    return out
